# revision 4
# baseline (speedup 1.0000x reference)
"""Trainium2 Bass kernel for a pre-LN transformer block (nn_Block_42339787604393).

Strategy: data-parallel over batch (8 batch elements -> 8 NeuronCores).
Each core computes the full block for one [1024, 768] batch element, entirely
in feature-major layout (features on partitions) so every matmul is a plain
K=128 contraction:

  qkv^T = W^T @ h^T          (lhsT = weight column-block, rhs = h^T)
  S^T_h = kpad_h^T.T @ q^T   (k zero-padded per head via host weight layout)
  P^T   = exp(0.125 * S^T)   (softmax without max-shift: |scores|*0.125 < ~3)
  out_h^T = v_aug^T.T @ P^T  (v augmented with a ones column -> rowsums)
  normalized via reciprocal + selector-matmul partition broadcast.

Host side pre-transposes x, pre-folds LayerNorm affines into the weights, and
transposes the [768, 1024] per-core output back.
"""
import numpy as np

import concourse.bass as bass
import concourse.mybir as mybir
import concourse.tile as tile
from concourse import bacc
from concourse.bass_utils import run_bass_kernel_spmd

F32 = mybir.dt.float32
AF = mybir.ActivationFunctionType
ALU = mybir.AluOpType

B = 8
N = 1024          # tokens
C = 768           # embed
H = 12            # heads
D = 64            # head dim
HID = 3072        # mlp hidden
KT = C // 128     # 6 embed k-tiles
MT = N // 128     # 8 token tiles
NH = N // 512     # 2 moving-operand halves
EPS = 1e-5
SCALING = D ** -0.5

_CACHED = {}


def _ln_feature_major(nc, src, dst, ps, sqp, skp, bcp, sel0, ones_col):
    """LayerNorm over the partition (feature) axis of src [128, KT, 1024]."""
    ps_s = ps.tile([1, N], F32, tag="stats", name="ps_s")
    ps_q = ps.tile([1, N], F32, tag="stats", name="ps_q")
    for kk in range(KT):
        for nh in range(NH):
            sl = slice(nh * 512, nh * 512 + 512)
            nc.tensor.matmul(ps_s[:, sl], lhsT=ones_col, rhs=src[:, kk, sl],
                             start=(kk == 0), stop=(kk == KT - 1))
        sq = sqp.tile([128, N], F32, tag="sq", name="sq")
        nc.scalar.activation(out=sq, in_=src[:, kk, :], func=AF.Square)
        for nh in range(NH):
            sl = slice(nh * 512, nh * 512 + 512)
            nc.tensor.matmul(ps_q[:, sl], lhsT=ones_col, rhs=sq[:, sl],
                             start=(kk == 0), stop=(kk == KT - 1))

    # stat tiles whose row 0 is broadcast via sel0; other rows must be finite
    stat_r = skp.tile([128, N], F32, tag="stat_r", name="stat_r")
    nc.vector.memset(stat_r, 0.0)
    stat_m = skp.tile([128, N], F32, tag="stat_m", name="stat_m")
    nc.vector.memset(stat_m, 0.0)

    mu = skp.tile([1, N], F32, tag="sk_a", name="mu")
    nc.vector.tensor_scalar_mul(mu, ps_s, 1.0 / C)
    ms = skp.tile([1, N], F32, tag="sk_b", name="ms")
    nc.vector.tensor_scalar_mul(ms, ps_q, 1.0 / C)
    vpe = skp.tile([1, N], F32, tag="sk_c", name="vpe")
    # vpe = -(mu*mu) + ms
    nc.vector.scalar_tensor_tensor(out=vpe, in0=mu, scalar=-1.0, in1=mu,
                                   op0=ALU.mult, op1=ALU.mult)
    nc.vector.tensor_tensor(out=vpe, in0=vpe, in1=ms, op=ALU.add)
    nc.vector.tensor_scalar_add(vpe, vpe, EPS)
    sd = skp.tile([1, N], F32, tag="sk_d", name="sd")
    nc.scalar.activation(out=sd, in_=vpe, func=AF.Sqrt)
    r0 = skp.tile([1, N], F32, tag="sk_e", name="r0")
    nc.vector.reciprocal(r0, sd)
    # one Newton step: r = r0 * (1.5 - 0.5 * vpe * r0^2)
    t1 = skp.tile([1, N], F32, tag="sk_f", name="t1")
    nc.vector.scalar_tensor_tensor(out=t1, in0=r0, scalar=-0.5, in1=r0,
                                   op0=ALU.mult, op1=ALU.mult)
    nc.vector.tensor_tensor(out=t1, in0=t1, in1=vpe, op=ALU.mult)
    nc.vector.tensor_scalar_add(t1, t1, 1.5)
    nc.vector.tensor_tensor(out=stat_r[0:1, :], in0=t1, in1=r0, op=ALU.mult)
    nc.vector.tensor_tensor(out=stat_m[0:1, :], in0=mu, in1=stat_r[0:1, :],
                            op=ALU.mult)

    # broadcast rsig / musig across partitions via selector matmul
    ps_r = ps.tile([128, N], F32, tag="mm", name="ps_r")
    ps_m = ps.tile([128, N], F32, tag="mm", name="ps_m")
    for nh in range(NH):
        sl = slice(nh * 512, nh * 512 + 512)
        nc.tensor.matmul(ps_r[:, sl], lhsT=sel0, rhs=stat_r[:, sl],
                         start=True, stop=True)
        nc.tensor.matmul(ps_m[:, sl], lhsT=sel0, rhs=stat_m[:, sl],
                         start=True, stop=True)
    rsig_b = bcp.tile([128, N], F32, tag="bc", name="rsig_b")
    nc.vector.tensor_copy(rsig_b, ps_r)
    musig_b = bcp.tile([128, N], F32, tag="bc", name="musig_b")
    nc.vector.tensor_copy(musig_b, ps_m)

    for kk in range(KT):
        tmp = bcp.tile([128, N], F32, tag="lt", name="lt")
        nc.vector.tensor_tensor(out=tmp, in0=src[:, kk, :], in1=rsig_b, op=ALU.mult)
        nc.vector.tensor_tensor(out=dst[:, kk, :], in0=tmp, in1=musig_b,
                                op=ALU.subtract)


def build_module(debug=False):
    nc = bacc.Bacc(None, target_bir_lowering=False)

    xT = nc.dram_tensor("xT", [C, N], F32, kind="ExternalInput")
    wq = nc.dram_tensor("wq", [C, C], F32, kind="ExternalInput")
    wkp = nc.dram_tensor("wkp", [C, H, 128], F32, kind="ExternalInput")
    wv = nc.dram_tensor("wv", [C, C], F32, kind="ExternalInput")
    wp = nc.dram_tensor("wp", [C, C], F32, kind="ExternalInput")
    wf1 = nc.dram_tensor("wf1", [C, HID], F32, kind="ExternalInput")
    wf2 = nc.dram_tensor("wf2", [HID, C], F32, kind="ExternalInput")
    bq_t = nc.dram_tensor("bq_t", [128, KT], F32, kind="ExternalInput")
    bkp_t = nc.dram_tensor("bkp_t", [128, H], F32, kind="ExternalInput")
    bv = nc.dram_tensor("bv", [C], F32, kind="ExternalInput")
    bp_t = nc.dram_tensor("bp_t", [128, KT], F32, kind="ExternalInput")
    bf1_t = nc.dram_tensor("bf1_t", [128, HID // 128], F32, kind="ExternalInput")
    bf2_t = nc.dram_tensor("bf2_t", [128, KT], F32, kind="ExternalInput")
    outT = nc.dram_tensor("outT", [C, N], F32, kind="ExternalOutput")
    if debug:
        d_hT = nc.dram_tensor("d_hT", [C, N], F32, kind="ExternalOutput")
        d_qT0 = nc.dram_tensor("d_qT0", [128, N], F32, kind="ExternalOutput")
        d_kp0 = nc.dram_tensor("d_kp0", [128, N], F32, kind="ExternalOutput")
        d_kp1 = nc.dram_tensor("d_kp1", [128, N], F32, kind="ExternalOutput")
        d_v = nc.dram_tensor("d_v", [128, 8, KT, 193], F32, kind="ExternalOutput")
        d_att = nc.dram_tensor("d_att", [C, N], F32, kind="ExternalOutput")
        d_x2 = nc.dram_tensor("d_x2", [C, N], F32, kind="ExternalOutput")
        d_h2 = nc.dram_tensor("d_h2", [C, N], F32, kind="ExternalOutput")

    with tile.TileContext(nc) as tc:
        with (
            tc.tile_pool(name="persist", bufs=1) as pers,
            tc.tile_pool(name="wpool", bufs=3) as wpl,
        ):
            # ---- constants ----
            sel0 = pers.tile([128, 128], F32, tag="sel0")
            nc.vector.memset(sel0, 0.0)
            nc.vector.memset(sel0[0:1, :], 1.0)
            sel_pair = pers.tile([128, 128], F32, tag="selp")
            nc.vector.memset(sel_pair, 0.0)
            nc.vector.memset(sel_pair[64:65, 0:64], 1.0)
            nc.vector.memset(sel_pair[0:1, 64:128], 1.0)
            ones_col = pers.tile([128, 1], F32, tag="ones")
            nc.vector.memset(ones_col, 1.0)
            bq_sb = pers.tile([128, KT], F32, tag="bq")
            nc.sync.dma_start(out=bq_sb, in_=bq_t[:, :])
            bkp_sb = pers.tile([128, H], F32, tag="bkp")
            nc.sync.dma_start(out=bkp_sb, in_=bkp_t[:, :])
            bp_sb = pers.tile([128, KT], F32, tag="bp")
            nc.sync.dma_start(out=bp_sb, in_=bp_t[:, :])
            bf1_sb = pers.tile([128, HID // 128], F32, tag="bf1")
            nc.sync.dma_start(out=bf1_sb, in_=bf1_t[:, :])
            bf2_sb = pers.tile([128, KT], F32, tag="bf2")
            nc.sync.dma_start(out=bf2_sb, in_=bf2_t[:, :])
            bv_b = pers.tile([128, C], F32, tag="bvb")
            nc.gpsimd.dma_start(
                out=bv_b,
                in_=bass.AP(tensor=bv[:].tensor, offset=0, ap=[[0, 128], [1, C]]))

            attnU = pers.tile([128, KT, N], F32, tag="att", name="attnU")

            with tc.tile_pool(name="hpool", bufs=1) as hpl, \
                 tc.tile_pool(name="vpool", bufs=1) as vpl:
                hT = hpl.tile([128, KT, N], F32, tag="h", name="hT")

                # ---------- phase A: LN1 + v ----------
                with tc.tile_pool(name="psA", bufs=2, space="PSUM") as psA:
                    with tc.tile_pool(name="ln1x", bufs=1) as xpl, \
                         tc.tile_pool(name="ln1sq", bufs=2) as sqp, \
                         tc.tile_pool(name="ln1sk", bufs=1) as skp, \
                         tc.tile_pool(name="ln1bc", bufs=2) as bcp:
                        xT_sb = xpl.tile([128, KT, N], F32, tag="x", name="xT_sb")
                        nc.sync.dma_start(
                            out=xT_sb, in_=xT[:, :].rearrange("(a p) n -> p a n", p=128))
                        _ln_feature_major(nc, xT_sb, hT, psA, sqp, skp, bcp,
                                          sel0, ones_col)

                    # v production (token-major, head slots with ones/zeros)
                    v_sb = vpl.tile([128, MT, KT, 193], F32, tag="v", name="v_sb")
                    nc.vector.memset(v_sb, 0.0)
                    nc.vector.memset(v_sb[:, :, :, 64:66], 1.0)
                    with tc.tile_pool(name="wvp", bufs=1) as wvp:
                        wv_sb = wvp.tile([128, KT, C], F32, tag="wv", name="wv_sb")
                        nc.sync.dma_start(
                            out=wv_sb, in_=wv[:, :].rearrange("(a p) m -> p a m", p=128))
                        bv_even = bv_b.rearrange("p (j two m) -> p j two m", two=2, m=D)
                        for t in range(MT):
                            # two bank-aligned psum tiles: an MM region must not
                            # cross a 512-f32 PSUM bank boundary
                            for n2 in range(2):
                                sl = slice(n2 * 384, n2 * 384 + 384)
                                ps_v = psA.tile([128, 384], F32, tag="mm",
                                                name=f"ps_v{n2}")
                                for kk in range(KT):
                                    nc.tensor.matmul(
                                        ps_v,
                                        lhsT=hT[:, kk, t * 128:(t + 1) * 128],
                                        rhs=wv_sb[:, kk, sl],
                                        start=(kk == 0), stop=(kk == KT - 1))
                                pv_view = ps_v.rearrange(
                                    "p (j two m) -> p j two m", two=2, m=D)
                                js = slice(n2 * 3, n2 * 3 + 3)
                                nc.vector.scalar_tensor_tensor(
                                    out=v_sb[:, t, js, 0:D],
                                    in0=pv_view[:, :, 0, :],
                                    scalar=0.0, in1=bv_even[:, js, 0, :],
                                    op0=ALU.add, op1=ALU.add)
                                nc.vector.scalar_tensor_tensor(
                                    out=v_sb[:, t, js, 129:193],
                                    in0=pv_view[:, :, 1, :],
                                    scalar=0.0, in1=bv_even[:, js, 1, :],
                                    op0=ALU.add, op1=ALU.add)

                if debug:
                    nc.sync.dma_start(
                        out=d_hT[:, :].rearrange("(a p) n -> p a n", p=128), in_=hT)
                    nc.sync.dma_start(out=d_v[:, :, :, :], in_=v_sb)
                # ---------- phase B: attention ----------
                with tc.tile_pool(name="psB", bufs=1, space="PSUM") as psB, \
                     tc.tile_pool(name="qkp", bufs=1) as qkp, \
                     tc.tile_pool(name="ppool", bufs=4) as ppl, \
                     tc.tile_pool(name="nrm", bufs=1) as nrm:
                    rec = nrm.tile([128, N], F32, tag="rec", name="rec")
                    nc.vector.memset(rec, 0.0)
                    for j in range(KT):  # 6 head pairs
                        # qT_j
                        wq_sb = wpl.tile([128, KT, 128], F32, tag="w", name="wq_sb")
                        nc.sync.dma_start(
                            out=wq_sb,
                            in_=wq[:, j * 128:(j + 1) * 128].rearrange(
                                "(a p) m -> p a m", p=128))
                        ps_qj = psB.tile([128, N], F32, tag="st", bufs=2, name="ps_qj")
                        for nh in range(NH):
                            sl = slice(nh * 512, nh * 512 + 512)
                            for kk in range(KT):
                                nc.tensor.matmul(ps_qj[:, sl], lhsT=wq_sb[:, kk, :],
                                                 rhs=hT[:, kk, sl],
                                                 start=(kk == 0), stop=(kk == KT - 1))
                        qT_j = qkp.tile([128, N], F32, tag="q", bufs=2, name="qT_j")
                        nc.vector.tensor_scalar_add(qT_j, ps_qj, bq_sb[:, j:j + 1])
                        if debug and j == 0:
                            nc.sync.dma_start(out=d_qT0[:, :], in_=qT_j)
                        # kpad for both heads of the pair
                        kpads = []
                        for par in range(2):
                            h = 2 * j + par
                            wk_sb = wpl.tile([128, KT, 128], F32, tag="w", name="wk_sb")
                            nc.sync.dma_start(
                                out=wk_sb,
                                in_=wkp[:, h, :].rearrange("(a p) m -> p a m", p=128))
                            ps_kj = psB.tile([128, N], F32, tag="st", bufs=2,
                                             name="ps_kj")
                            for nh in range(NH):
                                sl = slice(nh * 512, nh * 512 + 512)
                                for kk in range(KT):
                                    nc.tensor.matmul(ps_kj[:, sl], lhsT=wk_sb[:, kk, :],
                                                     rhs=hT[:, kk, sl],
                                                     start=(kk == 0),
                                                     stop=(kk == KT - 1))
                            kpad = qkp.tile([128, N], F32, tag="kp", bufs=4,
                                            name="kpad")
                            nc.vector.tensor_scalar_add(kpad, ps_kj,
                                                        bkp_sb[:, h:h + 1])
                            kpads.append(kpad)
                            if debug and j == 0:
                                nc.sync.dma_start(
                                    out=(d_kp0 if par == 0 else d_kp1)[:, :],
                                    in_=kpad)

                        # S^T + exp per token m-tile
                        ptiles = []
                        for mt in range(MT):
                            ps_s0 = psB.tile([128, N], F32, tag="st", bufs=2,
                                             name="ps_s0")
                            ps_s1 = psB.tile([128, N], F32, tag="st", bufs=2,
                                             name="ps_s1")
                            for nh in range(NH):
                                sl = slice(nh * 512, nh * 512 + 512)
                                nc.tensor.matmul(
                                    ps_s0[:, sl],
                                    lhsT=kpads[0][:, mt * 128:(mt + 1) * 128],
                                    rhs=qT_j[:, sl], start=True, stop=True)
                                nc.tensor.matmul(
                                    ps_s1[:, sl],
                                    lhsT=kpads[1][:, mt * 128:(mt + 1) * 128],
                                    rhs=qT_j[:, sl], start=True, stop=True)
                            p_t = ppl.tile([128, 2, N], F32, tag="p", name="p_t")
                            nc.scalar.activation(out=p_t[:, 0, :], in_=ps_s0,
                                                 func=AF.Exp, scale=SCALING)
                            nc.scalar.activation(out=p_t[:, 1, :], in_=ps_s1,
                                                 func=AF.Exp, scale=SCALING)
                            ptiles.append(p_t)

                        # PV (accumulate over token m-tiles)
                        pv0 = psB.tile([65, N], F32, tag="pv0", bufs=1, name="pv0")
                        pv1 = psB.tile([128, N], F32, tag="pv1", bufs=1, name="pv1")
                        for mt in range(MT):
                            for nh in range(NH):
                                sl = slice(nh * 512, nh * 512 + 512)
                                nc.tensor.matmul(pv0[:, sl],
                                                 lhsT=v_sb[:, mt, j, 0:65],
                                                 rhs=ptiles[mt][:, 0, sl],
                                                 start=(mt == 0), stop=(mt == MT - 1))
                                nc.tensor.matmul(pv1[:, sl],
                                                 lhsT=v_sb[:, mt, j, 65:193],
                                                 rhs=ptiles[mt][:, 1, sl],
                                                 start=(mt == 0), stop=(mt == MT - 1))

                        # normalization
                        nc.vector.reciprocal(rec[64:65, :], pv0[64:65, :])
                        nc.vector.reciprocal(rec[0:1, :], pv1[0:1, :])
                        for nh in range(NH):
                            sl = slice(nh * 512, nh * 512 + 512)
                            ps_rb = psB.tile([128, 512], F32, tag="st", bufs=2,
                                             name="ps_rb")
                            nc.tensor.matmul(ps_rb, lhsT=sel_pair, rhs=rec[:, sl],
                                             start=True, stop=True)
                            rb_sb = nrm.tile([128, 512], F32, tag="rb", bufs=2,
                                             name="rb_sb")
                            nc.vector.tensor_copy(rb_sb, ps_rb)
                            nc.vector.tensor_tensor(out=attnU[0:64, j, sl],
                                                    in0=pv0[0:64, sl],
                                                    in1=rb_sb[0:64, :], op=ALU.mult)
                            nc.vector.tensor_tensor(out=attnU[64:128, j, sl],
                                                    in0=pv1[64:128, sl],
                                                    in1=rb_sb[64:128, :], op=ALU.mult)

            if debug:
                nc.sync.dma_start(
                    out=d_att[:, :].rearrange("(a p) n -> p a n", p=128), in_=attnU)
            # ---------- phase C: proj + residual + LN2 ----------
            with tc.tile_pool(name="cdpool", bufs=1) as cdp:
                x2T = cdp.tile([128, KT, N], F32, tag="x2", name="x2T")
                h2T = cdp.tile([128, KT, N], F32, tag="h2", name="h2T")
                with tc.tile_pool(name="psC", bufs=2, space="PSUM") as psC:
                    with tc.tile_pool(name="xre", bufs=1) as xre:
                        xT_re = xre.tile([128, KT, N], F32, tag="xr", name="xT_re")
                        nc.sync.dma_start(
                            out=xT_re, in_=xT[:, :].rearrange("(a p) n -> p a n", p=128))
                        for m in range(KT):
                            wp_sb = wpl.tile([128, KT, 128], F32, tag="w", name="wp_sb")
                            nc.sync.dma_start(
                                out=wp_sb,
                                in_=wp[:, m * 128:(m + 1) * 128].rearrange(
                                    "(a p) m2 -> p a m2", p=128))
                            ps_p = psC.tile([128, N], F32, tag="mm", name="ps_p")
                            for nh in range(NH):
                                sl = slice(nh * 512, nh * 512 + 512)
                                for kk in range(KT):
                                    nc.tensor.matmul(ps_p[:, sl], lhsT=wp_sb[:, kk, :],
                                                     rhs=attnU[:, kk, sl],
                                                     start=(kk == 0),
                                                     stop=(kk == KT - 1))
                            nc.vector.scalar_tensor_tensor(
                                out=x2T[:, m, :], in0=ps_p, scalar=bp_sb[:, m:m + 1],
                                in1=xT_re[:, m, :], op0=ALU.add, op1=ALU.add)
                    if debug:
                        nc.sync.dma_start(
                            out=d_x2[:, :].rearrange("(a p) n -> p a n", p=128),
                            in_=x2T)
                    with tc.tile_pool(name="ln2sq", bufs=2) as sqp2, \
                         tc.tile_pool(name="ln2sk", bufs=1) as skp2, \
                         tc.tile_pool(name="ln2bc", bufs=2) as bcp2:
                        _ln_feature_major(nc, x2T, h2T, psC, sqp2, skp2, bcp2,
                                          sel0, ones_col)

                if debug:
                    nc.sync.dma_start(
                        out=d_h2[:, :].rearrange("(a p) n -> p a n", p=128), in_=h2T)
                # ---------- phase D: MLP ----------
                with tc.tile_pool(name="psD", bufs=2, space="PSUM") as psD, \
                     tc.tile_pool(name="h3p", bufs=12) as h3p, \
                     tc.tile_pool(name="w2p", bufs=2) as w2p, \
                     tc.tile_pool(name="outp", bufs=2) as outp:
                    mlp_acc = pers.tile([128, KT, N], F32, tag="att", name="mlp_acc")
                    for g in range(2):
                        h3_tiles = []
                        for i in range(12):
                            m1 = g * 12 + i
                            wf1_sb = wpl.tile([128, KT, 128], F32, tag="w",
                                              name="wf1_sb")
                            nc.sync.dma_start(
                                out=wf1_sb,
                                in_=wf1[:, m1 * 128:(m1 + 1) * 128].rearrange(
                                    "(a p) m -> p a m", p=128))
                            ps_f1 = psD.tile([128, N], F32, tag="f1", name="ps_f1")
                            for nh in range(NH):
                                sl = slice(nh * 512, nh * 512 + 512)
                                for kk in range(KT):
                                    nc.tensor.matmul(ps_f1[:, sl],
                                                     lhsT=wf1_sb[:, kk, :],
                                                     rhs=h2T[:, kk, sl],
                                                     start=(kk == 0),
                                                     stop=(kk == KT - 1))
                            h3_i = h3p.tile([128, N], F32, tag="h3", name="h3_i")
                            nc.scalar.activation(out=h3_i, in_=ps_f1, func=AF.Gelu,
                                                 bias=bf1_sb[:, m1:m1 + 1])
                            h3_tiles.append(h3_i)
                        for m2 in range(KT):
                            wf2_sb = w2p.tile([128, 12, 128], F32, tag="w2",
                                              name="wf2_sb")
                            nc.sync.dma_start(
                                out=wf2_sb,
                                in_=wf2[g * 1536:(g + 1) * 1536,
                                        m2 * 128:(m2 + 1) * 128].rearrange(
                                    "(a p) m -> p a m", p=128))
                            ps_f2 = psD.tile([128, N], F32, tag="f2", name="ps_f2")
                            for nh in range(NH):
                                sl = slice(nh * 512, nh * 512 + 512)
                                for k2 in range(12):
                                    nc.tensor.matmul(ps_f2[:, sl],
                                                     lhsT=wf2_sb[:, k2, :],
                                                     rhs=h3_tiles[k2][:, sl],
                                                     start=(k2 == 0), stop=(k2 == 11))
                            if g == 0:
                                nc.vector.scalar_tensor_tensor(
                                    out=mlp_acc[:, m2, :], in0=ps_f2,
                                    scalar=bf2_sb[:, m2:m2 + 1],
                                    in1=x2T[:, m2, :], op0=ALU.add, op1=ALU.add)
                            else:
                                out_t = outp.tile([128, N], F32, tag="o", name="out_t")
                                nc.vector.tensor_tensor(out=out_t, in0=ps_f2,
                                                        in1=mlp_acc[:, m2, :],
                                                        op=ALU.add)
                                nc.sync.dma_start(
                                    out=outT[m2 * 128:(m2 + 1) * 128, :], in_=out_t)
    nc.compile()
    return nc


def _prep_host_inputs(x, g1, b1, g2, b2, w_qkv, w_proj, b_proj, w_fc1, b_fc1,
                      w_fc2, b_fc2):
    f32 = np.float32
    g1 = np.asarray(g1, f32); b1 = np.asarray(b1, f32)
    g2 = np.asarray(g2, f32); b2 = np.asarray(b2, f32)
    w_qkv = np.asarray(w_qkv, f32)
    wq_full = g1[:, None] * w_qkv[:, 0:C]
    wk_full = g1[:, None] * w_qkv[:, C:2 * C]
    wv_full = g1[:, None] * w_qkv[:, 2 * C:3 * C]
    bqkv = b1 @ w_qkv
    bq, bk, bvv = bqkv[0:C], bqkv[C:2 * C], bqkv[2 * C:3 * C]

    wkp = np.zeros((C, H, 128), f32)
    bkp_t = np.zeros((128, H), f32)
    for h in range(H):
        off = 0 if h % 2 == 0 else 64
        wkp[:, h, off:off + D] = wk_full[:, h * D:(h + 1) * D]
        bkp_t[off:off + D, h] = bk[h * D:(h + 1) * D]

    def col_t(v):  # [k*128] -> [128, k] with [p, i] = v[i*128 + p]
        return np.ascontiguousarray(v.reshape(-1, 128).T).astype(f32)

    wf1_full = g2[:, None] * np.asarray(w_fc1, f32)
    bf1 = b2 @ np.asarray(w_fc1, f32) + np.asarray(b_fc1, f32)

    common = {
        "wq": np.ascontiguousarray(wq_full),
        "wkp": np.ascontiguousarray(wkp),
        "wv": np.ascontiguousarray(wv_full),
        "wp": np.ascontiguousarray(np.asarray(w_proj, f32)),
        "wf1": np.ascontiguousarray(wf1_full),
        "wf2": np.ascontiguousarray(np.asarray(w_fc2, f32)),
        "bq_t": col_t(bq),
        "bkp_t": np.ascontiguousarray(bkp_t),
        "bv": np.ascontiguousarray(bvv),
        "bp_t": col_t(np.asarray(b_proj, f32)),
        "bf1_t": col_t(bf1),
        "bf2_t": col_t(np.asarray(b_fc2, f32)),
    }
    x = np.asarray(x, f32)
    in_maps = []
    for b in range(B):
        m = dict(common)
        m["xT"] = np.ascontiguousarray(x[b].T)
        in_maps.append(m)
    return in_maps


def _run(inputs, trace=False):
    if "nc" not in _CACHED:
        _CACHED["nc"] = build_module()
    nc = _CACHED["nc"]
    in_maps = _prep_host_inputs(**inputs)
    res = run_bass_kernel_spmd(nc, in_maps, core_ids=list(range(B)), trace=trace)
    out = np.stack([np.asarray(r["outT"]).T for r in res.results])
    return out.astype(np.float32), res


def kernel(**inputs):
    out, _ = _run(inputs, trace=False)
    return out


# revision 7
# speedup vs baseline: 2.9917x; 2.9917x over previous
"""Trainium2 Bass kernel for a pre-LN transformer block (nn_Block_42339787604393).

Strategy: data-parallel over batch (8 batch elements -> 8 NeuronCores).
Each core computes the full block for one [1024, 768] batch element, entirely
in feature-major layout (features on partitions) so every matmul is a plain
K=128 contraction:

  qkv^T = W^T @ h^T          (lhsT = weight column-block, rhs = h^T)
  S^T_h = kpad_h^T.T @ q^T   (k zero-padded per head via host weight layout)
  P^T   = exp(0.125 * S^T)   (softmax without max-shift: |scores|*0.125 < ~3)
  out_h^T = v_aug^T.T @ P^T  (v augmented with a ones column -> rowsums)
  normalized via reciprocal + selector-matmul partition broadcast.

Host side pre-transposes x, pre-folds LayerNorm affines into the weights, and
transposes the [768, 1024] per-core output back.
"""
import numpy as np

import concourse.bass as bass
import concourse.mybir as mybir
import concourse.tile as tile
from concourse import bacc
from concourse.bass_utils import run_bass_kernel_spmd

F32 = mybir.dt.float32
USE_F32R = True
MMDT = mybir.dt.float32r if USE_F32R else F32
AF = mybir.ActivationFunctionType
ALU = mybir.AluOpType

B = 8
N = 1024          # tokens
C = 768           # embed
H = 12            # heads
D = 64            # head dim
HID = 3072        # mlp hidden
KT = C // 128     # 6 embed k-tiles
MT = N // 128     # 8 token tiles
NH = N // 512     # 2 moving-operand halves
EPS = 1e-5
SCALING = D ** -0.5

_CACHED = {}


def _memset(nc, ap, val):
    # memset can't encode float32r; write the same bits through an f32 view
    if ap.dtype == mybir.dt.float32r:
        ap = ap.bitcast(F32)
    nc.vector.memset(ap, val)


def _ln_feature_major(nc, src, dst, ps, sqp, skp, bcp, sel0, ones_col):
    """LayerNorm over the partition (feature) axis of src [128, KT, 1024]."""
    ps_s = ps.tile([1, N], F32, tag="stats", name="ps_s")
    ps_q = ps.tile([1, N], F32, tag="stats", name="ps_q")
    for kk in range(KT):
        for nh in range(NH):
            sl = slice(nh * 512, nh * 512 + 512)
            nc.tensor.matmul(ps_s[:, sl], lhsT=ones_col, rhs=src[:, kk, sl],
                             start=(kk == 0), stop=(kk == KT - 1))
        sq = sqp.tile([128, N], MMDT, tag="sq", name="sq")
        nc.scalar.activation(out=sq, in_=src[:, kk, :], func=AF.Square)
        for nh in range(NH):
            sl = slice(nh * 512, nh * 512 + 512)
            nc.tensor.matmul(ps_q[:, sl], lhsT=ones_col, rhs=sq[:, sl],
                             start=(kk == 0), stop=(kk == KT - 1))

    # stat tiles whose row 0 is broadcast via sel0; other rows must be finite
    stat_r = skp.tile([128, N], MMDT, tag="stat_r", name="stat_r")
    _memset(nc, stat_r, 0.0)
    stat_m = skp.tile([128, N], MMDT, tag="stat_m", name="stat_m")
    _memset(nc, stat_m, 0.0)

    mu = skp.tile([1, N], F32, tag="sk_a", name="mu")
    nc.vector.tensor_scalar_mul(mu, ps_s, 1.0 / C)
    ms = skp.tile([1, N], F32, tag="sk_b", name="ms")
    nc.vector.tensor_scalar_mul(ms, ps_q, 1.0 / C)
    vpe = skp.tile([1, N], F32, tag="sk_c", name="vpe")
    # vpe = -(mu*mu) + ms
    nc.vector.scalar_tensor_tensor(out=vpe, in0=mu, scalar=-1.0, in1=mu,
                                   op0=ALU.mult, op1=ALU.mult)
    nc.vector.tensor_tensor(out=vpe, in0=vpe, in1=ms, op=ALU.add)
    nc.vector.tensor_scalar_add(vpe, vpe, EPS)
    sd = skp.tile([1, N], F32, tag="sk_d", name="sd")
    nc.scalar.activation(out=sd, in_=vpe, func=AF.Sqrt)
    r0 = skp.tile([1, N], F32, tag="sk_e", name="r0")
    nc.vector.reciprocal(r0, sd)
    # one Newton step: r = r0 * (1.5 - 0.5 * vpe * r0^2)
    t1 = skp.tile([1, N], F32, tag="sk_f", name="t1")
    nc.vector.scalar_tensor_tensor(out=t1, in0=r0, scalar=-0.5, in1=r0,
                                   op0=ALU.mult, op1=ALU.mult)
    nc.vector.tensor_tensor(out=t1, in0=t1, in1=vpe, op=ALU.mult)
    nc.vector.tensor_scalar_add(t1, t1, 1.5)
    nc.vector.tensor_tensor(out=stat_r[0:1, :], in0=t1, in1=r0, op=ALU.mult)
    nc.vector.tensor_tensor(out=stat_m[0:1, :], in0=mu, in1=stat_r[0:1, :],
                            op=ALU.mult)

    # broadcast rsig / musig across partitions via selector matmul
    ps_r = ps.tile([128, N], F32, tag="mm", name="ps_r")
    ps_m = ps.tile([128, N], F32, tag="mm", name="ps_m")
    for nh in range(NH):
        sl = slice(nh * 512, nh * 512 + 512)
        nc.tensor.matmul(ps_r[:, sl], lhsT=sel0, rhs=stat_r[:, sl],
                         start=True, stop=True)
        nc.tensor.matmul(ps_m[:, sl], lhsT=sel0, rhs=stat_m[:, sl],
                         start=True, stop=True)
    rsig_b = bcp.tile([128, N], F32, tag="bc", name="rsig_b")
    nc.vector.tensor_copy(rsig_b, ps_r)
    musig_b = bcp.tile([128, N], F32, tag="bc", name="musig_b")
    nc.vector.tensor_copy(musig_b, ps_m)

    for kk in range(KT):
        tmp = bcp.tile([128, N], F32, tag="lt", name="lt")
        nc.vector.tensor_tensor(out=tmp, in0=src[:, kk, :], in1=rsig_b, op=ALU.mult)
        nc.vector.tensor_tensor(out=dst[:, kk, :], in0=tmp, in1=musig_b,
                                op=ALU.subtract)


def build_module(debug=False):
    nc = bacc.Bacc(None, target_bir_lowering=False)

    xT = nc.dram_tensor("xT", [C, N], MMDT, kind="ExternalInput")
    wq = nc.dram_tensor("wq", [C, C], MMDT, kind="ExternalInput")
    wkp = nc.dram_tensor("wkp", [C, H, 128], MMDT, kind="ExternalInput")
    wv = nc.dram_tensor("wv", [C, C], MMDT, kind="ExternalInput")
    wp = nc.dram_tensor("wp", [C, C], MMDT, kind="ExternalInput")
    wf1 = nc.dram_tensor("wf1", [C, HID], MMDT, kind="ExternalInput")
    wf2 = nc.dram_tensor("wf2", [HID, C], MMDT, kind="ExternalInput")
    bq_t = nc.dram_tensor("bq_t", [128, KT], F32, kind="ExternalInput")
    bkp_t = nc.dram_tensor("bkp_t", [128, H], F32, kind="ExternalInput")
    bv = nc.dram_tensor("bv", [C], F32, kind="ExternalInput")
    bp_t = nc.dram_tensor("bp_t", [128, KT], F32, kind="ExternalInput")
    bf1_t = nc.dram_tensor("bf1_t", [128, HID // 128], F32, kind="ExternalInput")
    bf2_t = nc.dram_tensor("bf2_t", [128, KT], F32, kind="ExternalInput")
    outT = nc.dram_tensor("outT", [C, N], F32, kind="ExternalOutput")
    if debug:
        d_hT = nc.dram_tensor("d_hT", [C, N], MMDT, kind="ExternalOutput")
        d_qT0 = nc.dram_tensor("d_qT0", [128, N], MMDT, kind="ExternalOutput")
        d_kp0 = nc.dram_tensor("d_kp0", [128, N], MMDT, kind="ExternalOutput")
        d_kp1 = nc.dram_tensor("d_kp1", [128, N], MMDT, kind="ExternalOutput")
        d_v = nc.dram_tensor("d_v", [128, 8, KT, 193], MMDT, kind="ExternalOutput")
        d_att = nc.dram_tensor("d_att", [C, N], MMDT, kind="ExternalOutput")
        d_x2 = nc.dram_tensor("d_x2", [C, N], MMDT, kind="ExternalOutput")
        d_h2 = nc.dram_tensor("d_h2", [C, N], MMDT, kind="ExternalOutput")

    with tile.TileContext(nc) as tc:
        with (
            tc.tile_pool(name="persist", bufs=1) as pers,
            tc.tile_pool(name="wpool", bufs=3) as wpl,
        ):
            # ---- constants ----
            sel0 = pers.tile([128, 128], MMDT, tag="sel0")
            _memset(nc, sel0, 0.0)
            _memset(nc, sel0[0:1, :], 1.0)
            sel_pair = pers.tile([128, 128], MMDT, tag="selp")
            _memset(nc, sel_pair, 0.0)
            _memset(nc, sel_pair[64:65, 0:64], 1.0)
            _memset(nc, sel_pair[0:1, 64:128], 1.0)
            ones_col = pers.tile([128, 1], MMDT, tag="ones")
            _memset(nc, ones_col, 1.0)
            bq_sb = pers.tile([128, KT], F32, tag="bq")
            nc.sync.dma_start(out=bq_sb, in_=bq_t[:, :])
            bkp_sb = pers.tile([128, H], F32, tag="bkp")
            nc.sync.dma_start(out=bkp_sb, in_=bkp_t[:, :])
            bp_sb = pers.tile([128, KT], F32, tag="bp")
            nc.sync.dma_start(out=bp_sb, in_=bp_t[:, :])
            bf1_sb = pers.tile([128, HID // 128], F32, tag="bf1")
            nc.sync.dma_start(out=bf1_sb, in_=bf1_t[:, :])
            bf2_sb = pers.tile([128, KT], F32, tag="bf2")
            nc.sync.dma_start(out=bf2_sb, in_=bf2_t[:, :])
            bv_b = pers.tile([128, C], F32, tag="bvb")
            nc.gpsimd.dma_start(
                out=bv_b,
                in_=bass.AP(tensor=bv[:].tensor, offset=0, ap=[[0, 128], [1, C]]))

            attnU = pers.tile([128, KT, N], MMDT, tag="att", name="attnU")

            with tc.tile_pool(name="hpool", bufs=1) as hpl, \
                 tc.tile_pool(name="vpool", bufs=1) as vpl:
                hT = hpl.tile([128, KT, N], MMDT, tag="h", name="hT")

                # ---------- phase A: LN1 + v ----------
                with tc.tile_pool(name="psA", bufs=2, space="PSUM") as psA:
                    with tc.tile_pool(name="ln1x", bufs=1) as xpl, \
                         tc.tile_pool(name="ln1sq", bufs=2) as sqp, \
                         tc.tile_pool(name="ln1sk", bufs=1) as skp, \
                         tc.tile_pool(name="ln1bc", bufs=2) as bcp:
                        xT_sb = xpl.tile([128, KT, N], MMDT, tag="x", name="xT_sb")
                        nc.sync.dma_start(
                            out=xT_sb, in_=xT[:, :].rearrange("(a p) n -> p a n", p=128))
                        _ln_feature_major(nc, xT_sb, hT, psA, sqp, skp, bcp,
                                          sel0, ones_col)

                    # v production (token-major, head slots with ones/zeros)
                    v_sb = vpl.tile([128, MT, KT, 193], MMDT, tag="v", name="v_sb")
                    _memset(nc, v_sb, 0.0)
                    _memset(nc, v_sb[:, :, :, 64:66], 1.0)
                    with tc.tile_pool(name="wvp", bufs=1) as wvp:
                        wv_sb = wvp.tile([128, KT, C], MMDT, tag="wv", name="wv_sb")
                        nc.sync.dma_start(
                            out=wv_sb, in_=wv[:, :].rearrange("(a p) m -> p a m", p=128))
                        bv_even = bv_b.rearrange("p (j two m) -> p j two m", two=2, m=D)
                        for t in range(MT):
                            # two bank-aligned psum tiles: an MM region must not
                            # cross a 512-f32 PSUM bank boundary
                            for n2 in range(2):
                                sl = slice(n2 * 384, n2 * 384 + 384)
                                ps_v = psA.tile([128, 384], F32, tag="mm",
                                                name=f"ps_v{n2}")
                                for kk in range(KT):
                                    nc.tensor.matmul(
                                        ps_v,
                                        lhsT=hT[:, kk, t * 128:(t + 1) * 128],
                                        rhs=wv_sb[:, kk, sl],
                                        start=(kk == 0), stop=(kk == KT - 1))
                                pv_view = ps_v.rearrange(
                                    "p (j two m) -> p j two m", two=2, m=D)
                                js = slice(n2 * 3, n2 * 3 + 3)
                                nc.vector.scalar_tensor_tensor(
                                    out=v_sb[:, t, js, 0:D],
                                    in0=pv_view[:, :, 0, :],
                                    scalar=0.0, in1=bv_even[:, js, 0, :],
                                    op0=ALU.add, op1=ALU.add)
                                nc.vector.scalar_tensor_tensor(
                                    out=v_sb[:, t, js, 129:193],
                                    in0=pv_view[:, :, 1, :],
                                    scalar=0.0, in1=bv_even[:, js, 1, :],
                                    op0=ALU.add, op1=ALU.add)

                if debug:
                    nc.sync.dma_start(
                        out=d_hT[:, :].rearrange("(a p) n -> p a n", p=128), in_=hT)
                    nc.sync.dma_start(out=d_v[:, :, :, :], in_=v_sb)
                # ---------- phase B: attention ----------
                with tc.tile_pool(name="psB", bufs=1, space="PSUM") as psB, \
                     tc.tile_pool(name="qkp", bufs=1) as qkp, \
                     tc.tile_pool(name="ppool", bufs=4) as ppl, \
                     tc.tile_pool(name="nrm", bufs=1) as nrm:
                    rec = nrm.tile([128, N], MMDT, tag="rec", name="rec")
                    _memset(nc, rec, 0.0)
                    for j in range(KT):  # 6 head pairs
                        # qT_j
                        wq_sb = wpl.tile([128, KT, 128], MMDT, tag="w", name="wq_sb")
                        nc.sync.dma_start(
                            out=wq_sb,
                            in_=wq[:, j * 128:(j + 1) * 128].rearrange(
                                "(a p) m -> p a m", p=128))
                        ps_qj = psB.tile([128, N], F32, tag="st", bufs=2, name="ps_qj")
                        for nh in range(NH):
                            sl = slice(nh * 512, nh * 512 + 512)
                            for kk in range(KT):
                                nc.tensor.matmul(ps_qj[:, sl], lhsT=wq_sb[:, kk, :],
                                                 rhs=hT[:, kk, sl],
                                                 start=(kk == 0), stop=(kk == KT - 1))
                        qT_j = qkp.tile([128, N], MMDT, tag="q", bufs=2, name="qT_j")
                        nc.vector.tensor_scalar_add(qT_j, ps_qj, bq_sb[:, j:j + 1])
                        if debug and j == 0:
                            nc.sync.dma_start(out=d_qT0[:, :], in_=qT_j)
                        # kpad for both heads of the pair
                        kpads = []
                        for par in range(2):
                            h = 2 * j + par
                            wk_sb = wpl.tile([128, KT, 128], MMDT, tag="w", name="wk_sb")
                            nc.sync.dma_start(
                                out=wk_sb,
                                in_=wkp[:, h, :].rearrange("(a p) m -> p a m", p=128))
                            ps_kj = psB.tile([128, N], F32, tag="st", bufs=2,
                                             name="ps_kj")
                            for nh in range(NH):
                                sl = slice(nh * 512, nh * 512 + 512)
                                for kk in range(KT):
                                    nc.tensor.matmul(ps_kj[:, sl], lhsT=wk_sb[:, kk, :],
                                                     rhs=hT[:, kk, sl],
                                                     start=(kk == 0),
                                                     stop=(kk == KT - 1))
                            kpad = qkp.tile([128, N], MMDT, tag="kp", bufs=4,
                                            name="kpad")
                            nc.vector.tensor_scalar_add(kpad, ps_kj,
                                                        bkp_sb[:, h:h + 1])
                            kpads.append(kpad)
                            if debug and j == 0:
                                nc.sync.dma_start(
                                    out=(d_kp0 if par == 0 else d_kp1)[:, :],
                                    in_=kpad)

                        # S^T + exp per token m-tile
                        ptiles = []
                        for mt in range(MT):
                            ps_s0 = psB.tile([128, N], F32, tag="st", bufs=2,
                                             name="ps_s0")
                            ps_s1 = psB.tile([128, N], F32, tag="st", bufs=2,
                                             name="ps_s1")
                            for nh in range(NH):
                                sl = slice(nh * 512, nh * 512 + 512)
                                nc.tensor.matmul(
                                    ps_s0[:, sl],
                                    lhsT=kpads[0][:, mt * 128:(mt + 1) * 128],
                                    rhs=qT_j[:, sl], start=True, stop=True)
                                nc.tensor.matmul(
                                    ps_s1[:, sl],
                                    lhsT=kpads[1][:, mt * 128:(mt + 1) * 128],
                                    rhs=qT_j[:, sl], start=True, stop=True)
                            p_t = ppl.tile([128, 2, N], MMDT, tag="p", name="p_t")
                            nc.scalar.activation(out=p_t[:, 0, :], in_=ps_s0,
                                                 func=AF.Exp, scale=SCALING)
                            nc.scalar.activation(out=p_t[:, 1, :], in_=ps_s1,
                                                 func=AF.Exp, scale=SCALING)
                            ptiles.append(p_t)

                        # PV (accumulate over token m-tiles)
                        pv0 = psB.tile([65, N], F32, tag="pv0", bufs=1, name="pv0")
                        pv1 = psB.tile([128, N], F32, tag="pv1", bufs=1, name="pv1")
                        for mt in range(MT):
                            for nh in range(NH):
                                sl = slice(nh * 512, nh * 512 + 512)
                                nc.tensor.matmul(pv0[:, sl],
                                                 lhsT=v_sb[:, mt, j, 0:65],
                                                 rhs=ptiles[mt][:, 0, sl],
                                                 start=(mt == 0), stop=(mt == MT - 1))
                                nc.tensor.matmul(pv1[:, sl],
                                                 lhsT=v_sb[:, mt, j, 65:193],
                                                 rhs=ptiles[mt][:, 1, sl],
                                                 start=(mt == 0), stop=(mt == MT - 1))

                        # normalization
                        with nc.allow_low_precision(
                                reason="f32r recip feeds f32r bcast matmul"):
                            nc.vector.reciprocal(rec[64:65, :], pv0[64:65, :])
                            nc.vector.reciprocal(rec[0:1, :], pv1[0:1, :])
                        for nh in range(NH):
                            sl = slice(nh * 512, nh * 512 + 512)
                            ps_rb = psB.tile([128, 512], F32, tag="st", bufs=2,
                                             name="ps_rb")
                            nc.tensor.matmul(ps_rb, lhsT=sel_pair, rhs=rec[:, sl],
                                             start=True, stop=True)
                            rb_sb = nrm.tile([128, 512], F32, tag="rb", bufs=2,
                                             name="rb_sb")
                            nc.vector.tensor_copy(rb_sb, ps_rb)
                            nc.vector.tensor_tensor(out=attnU[0:64, j, sl],
                                                    in0=pv0[0:64, sl],
                                                    in1=rb_sb[0:64, :], op=ALU.mult)
                            nc.vector.tensor_tensor(out=attnU[64:128, j, sl],
                                                    in0=pv1[64:128, sl],
                                                    in1=rb_sb[64:128, :], op=ALU.mult)

            if debug:
                nc.sync.dma_start(
                    out=d_att[:, :].rearrange("(a p) n -> p a n", p=128), in_=attnU)
            # ---------- phase C: proj + residual + LN2 ----------
            with tc.tile_pool(name="cdpool", bufs=1) as cdp:
                x2T = cdp.tile([128, KT, N], MMDT, tag="x2", name="x2T")
                h2T = cdp.tile([128, KT, N], MMDT, tag="h2", name="h2T")
                with tc.tile_pool(name="psC", bufs=2, space="PSUM") as psC:
                    with tc.tile_pool(name="xre", bufs=1) as xre:
                        xT_re = xre.tile([128, KT, N], MMDT, tag="xr", name="xT_re")
                        nc.sync.dma_start(
                            out=xT_re, in_=xT[:, :].rearrange("(a p) n -> p a n", p=128))
                        for m in range(KT):
                            wp_sb = wpl.tile([128, KT, 128], MMDT, tag="w", name="wp_sb")
                            nc.sync.dma_start(
                                out=wp_sb,
                                in_=wp[:, m * 128:(m + 1) * 128].rearrange(
                                    "(a p) m2 -> p a m2", p=128))
                            ps_p = psC.tile([128, N], F32, tag="mm", name="ps_p")
                            for nh in range(NH):
                                sl = slice(nh * 512, nh * 512 + 512)
                                for kk in range(KT):
                                    nc.tensor.matmul(ps_p[:, sl], lhsT=wp_sb[:, kk, :],
                                                     rhs=attnU[:, kk, sl],
                                                     start=(kk == 0),
                                                     stop=(kk == KT - 1))
                            nc.vector.scalar_tensor_tensor(
                                out=x2T[:, m, :], in0=ps_p, scalar=bp_sb[:, m:m + 1],
                                in1=xT_re[:, m, :], op0=ALU.add, op1=ALU.add)
                    if debug:
                        nc.sync.dma_start(
                            out=d_x2[:, :].rearrange("(a p) n -> p a n", p=128),
                            in_=x2T)
                    with tc.tile_pool(name="ln2sq", bufs=2) as sqp2, \
                         tc.tile_pool(name="ln2sk", bufs=1) as skp2, \
                         tc.tile_pool(name="ln2bc", bufs=2) as bcp2:
                        _ln_feature_major(nc, x2T, h2T, psC, sqp2, skp2, bcp2,
                                          sel0, ones_col)

                if debug:
                    nc.sync.dma_start(
                        out=d_h2[:, :].rearrange("(a p) n -> p a n", p=128), in_=h2T)
                # ---------- phase D: MLP ----------
                with tc.tile_pool(name="psD", bufs=2, space="PSUM") as psD, \
                     tc.tile_pool(name="h3p", bufs=12) as h3p, \
                     tc.tile_pool(name="w2p", bufs=2) as w2p, \
                     tc.tile_pool(name="outp", bufs=2) as outp:
                    mlp_acc = pers.tile([128, KT, N], F32, tag="att", name="mlp_acc")
                    for g in range(2):
                        h3_tiles = []
                        for i in range(12):
                            m1 = g * 12 + i
                            wf1_sb = wpl.tile([128, KT, 128], MMDT, tag="w",
                                              name="wf1_sb")
                            nc.sync.dma_start(
                                out=wf1_sb,
                                in_=wf1[:, m1 * 128:(m1 + 1) * 128].rearrange(
                                    "(a p) m -> p a m", p=128))
                            ps_f1 = psD.tile([128, N], F32, tag="f1", name="ps_f1")
                            for nh in range(NH):
                                sl = slice(nh * 512, nh * 512 + 512)
                                for kk in range(KT):
                                    nc.tensor.matmul(ps_f1[:, sl],
                                                     lhsT=wf1_sb[:, kk, :],
                                                     rhs=h2T[:, kk, sl],
                                                     start=(kk == 0),
                                                     stop=(kk == KT - 1))
                            h3_i = h3p.tile([128, N], MMDT, tag="h3", name="h3_i")
                            nc.scalar.activation(out=h3_i, in_=ps_f1, func=AF.Gelu,
                                                 bias=bf1_sb[:, m1:m1 + 1])
                            h3_tiles.append(h3_i)
                        for m2 in range(KT):
                            wf2_sb = w2p.tile([128, 12, 128], MMDT, tag="w2",
                                              name="wf2_sb")
                            nc.sync.dma_start(
                                out=wf2_sb,
                                in_=wf2[g * 1536:(g + 1) * 1536,
                                        m2 * 128:(m2 + 1) * 128].rearrange(
                                    "(a p) m -> p a m", p=128))
                            ps_f2 = psD.tile([128, N], F32, tag="f2", name="ps_f2")
                            for nh in range(NH):
                                sl = slice(nh * 512, nh * 512 + 512)
                                for k2 in range(12):
                                    nc.tensor.matmul(ps_f2[:, sl],
                                                     lhsT=wf2_sb[:, k2, :],
                                                     rhs=h3_tiles[k2][:, sl],
                                                     start=(k2 == 0), stop=(k2 == 11))
                            if g == 0:
                                nc.vector.scalar_tensor_tensor(
                                    out=mlp_acc[:, m2, :], in0=ps_f2,
                                    scalar=bf2_sb[:, m2:m2 + 1],
                                    in1=x2T[:, m2, :], op0=ALU.add, op1=ALU.add)
                            else:
                                out_t = outp.tile([128, N], F32, tag="o", name="out_t")
                                nc.vector.tensor_tensor(out=out_t, in0=ps_f2,
                                                        in1=mlp_acc[:, m2, :],
                                                        op=ALU.add)
                                nc.sync.dma_start(
                                    out=outT[m2 * 128:(m2 + 1) * 128, :], in_=out_t)
    nc.compile()
    return nc


def _prep_host_inputs(x, g1, b1, g2, b2, w_qkv, w_proj, b_proj, w_fc1, b_fc1,
                      w_fc2, b_fc2):
    f32 = np.float32
    g1 = np.asarray(g1, f32); b1 = np.asarray(b1, f32)
    g2 = np.asarray(g2, f32); b2 = np.asarray(b2, f32)
    w_qkv = np.asarray(w_qkv, f32)
    wq_full = g1[:, None] * w_qkv[:, 0:C]
    wk_full = g1[:, None] * w_qkv[:, C:2 * C]
    wv_full = g1[:, None] * w_qkv[:, 2 * C:3 * C]
    bqkv = b1 @ w_qkv
    bq, bk, bvv = bqkv[0:C], bqkv[C:2 * C], bqkv[2 * C:3 * C]

    wkp = np.zeros((C, H, 128), f32)
    bkp_t = np.zeros((128, H), f32)
    for h in range(H):
        off = 0 if h % 2 == 0 else 64
        wkp[:, h, off:off + D] = wk_full[:, h * D:(h + 1) * D]
        bkp_t[off:off + D, h] = bk[h * D:(h + 1) * D]

    def col_t(v):  # [k*128] -> [128, k] with [p, i] = v[i*128 + p]
        return np.ascontiguousarray(v.reshape(-1, 128).T).astype(f32)

    wf1_full = g2[:, None] * np.asarray(w_fc1, f32)
    bf1 = b2 @ np.asarray(w_fc1, f32) + np.asarray(b_fc1, f32)

    common = {
        "wq": np.ascontiguousarray(wq_full),
        "wkp": np.ascontiguousarray(wkp),
        "wv": np.ascontiguousarray(wv_full),
        "wp": np.ascontiguousarray(np.asarray(w_proj, f32)),
        "wf1": np.ascontiguousarray(wf1_full),
        "wf2": np.ascontiguousarray(np.asarray(w_fc2, f32)),
        "bq_t": col_t(bq),
        "bkp_t": np.ascontiguousarray(bkp_t),
        "bv": np.ascontiguousarray(bvv),
        "bp_t": col_t(np.asarray(b_proj, f32)),
        "bf1_t": col_t(bf1),
        "bf2_t": col_t(np.asarray(b_fc2, f32)),
    }
    x = np.asarray(x, f32)
    in_maps = []
    for b in range(B):
        m = dict(common)
        m["xT"] = np.ascontiguousarray(x[b].T)
        in_maps.append(m)
    return in_maps


def _run(inputs, trace=False):
    if "nc" not in _CACHED:
        _CACHED["nc"] = build_module()
    nc = _CACHED["nc"]
    in_maps = _prep_host_inputs(**inputs)
    res = run_bass_kernel_spmd(nc, in_maps, core_ids=list(range(B)), trace=trace)
    out = np.stack([np.asarray(r["outT"]).T for r in res.results])
    return out.astype(np.float32), res


def kernel(**inputs):
    out, _ = _run(inputs, trace=False)
    return out
